# revision 1
# baseline (speedup 1.0000x reference)
"""Cross-attention block kernel for Trainium2 (8 NeuronCores, data-parallel).

Computes, for full inputs:
    Q = x @ Wq + bq            [B, HW, D]
    K = a @ Wk + bk            [B, S, D]
    V = a @ Wv + bv            [B, S, D]
    out = softmax(Q K^T / sqrt(D)) @ V

Sharding: batch (B=16) split across 8 cores, 2 batches per core. Weights
replicated. No collectives needed.

Per-core kernel strategy (all matmuls in float32r = full PE rate, FP22):
  - PE-transpose x and audio tiles into d-major SBUF layouts (xT, aT).
  - qT = Wq^T @ xT   (out [d_out-part, hw-free]; bias added by ACT copy)
  - kT = Wk^T @ aT   (out [d_out-part, s-free])
  - v  = aT^T @ Wv   (out [s-part, d-free]; bias added by DVE)
  - scoresT[s, hw] = (kT chunk)^T @ qT, accumulated over d in PSUM;
    ACT computes exp(scale * scoresT) straight out of PSUM (no max
    subtraction: scores have std ~0.33, max |score| < ~3, exp is safe).
  - out[hw, d] = sum_s expT^T @ V with an interleaved ones-column matmul
    accumulating the softmax denominator in a second PSUM bank; the
    final ACT copy applies the reciprocal as a per-partition scale.

float32r is a 4-byte fp32 view that the PE truncates to FP22; the walrus
verifier requires every producer feeding an FP32r matmul to emit float32r,
so the whole operand chain (DRAM tensors included) is declared float32r.
"""

from contextlib import ExitStack

import numpy as np

import concourse.bass as bass
import concourse.bacc as bacc
import concourse.mybir as mybir
import concourse.tile as tile
from concourse.bass_utils import run_bass_kernel_spmd
from concourse.masks import make_identity

P = 128
D = 512          # d_query == d_audio == d_out
CD = D // P      # 4 chunks of the feature dim
HW = 4096        # queries per batch
S = 1024         # keys per batch
SC = S // P      # 8 s-chunks
HWB = 512        # hw rows processed per block
NBLK = HW // HWB
B_FULL = 16
N_CORES = 8
BL = B_FULL // N_CORES  # 2 batches per core
SCALE = 1.0 / float(np.sqrt(D))

f32 = mybir.dt.float32
f32r = mybir.dt.float32r
AFT = mybir.ActivationFunctionType


def build_nc():
    nc = bacc.Bacc("TRN2", target_bir_lowering=False, debug=False)

    x = nc.dram_tensor("x", [BL, HW, D], f32r, kind="ExternalInput").ap()
    audio = nc.dram_tensor("audio_embed", [BL, S, D], f32r, kind="ExternalInput").ap()
    wq = nc.dram_tensor("Wq", [D, D], f32r, kind="ExternalInput").ap()
    bq = nc.dram_tensor("bq", [D], f32, kind="ExternalInput").ap()
    wk = nc.dram_tensor("Wk", [D, D], f32r, kind="ExternalInput").ap()
    bk = nc.dram_tensor("bk", [D], f32, kind="ExternalInput").ap()
    wv = nc.dram_tensor("Wv", [D, D], f32r, kind="ExternalInput").ap()
    bv = nc.dram_tensor("bv", [D], f32r, kind="ExternalInput").ap()
    out = nc.dram_tensor("out", [BL, HW, D], f32, kind="ExternalOutput").ap()

    with tile.TileContext(nc) as tc:
        with ExitStack() as ctx:
            _body(ctx, tc, x, audio, wq, bq, wk, bk, wv, bv, out)

    nc.compile()
    return nc


def _body(ctx, tc, x, audio, wq, bq, wk, bk, wv, bv, out):
    nc = tc.nc

    const_pool = ctx.enter_context(tc.tile_pool(name="const", bufs=1))
    batch_pool = ctx.enter_context(tc.tile_pool(name="batch", bufs=1))
    work_pool = ctx.enter_context(tc.tile_pool(name="work", bufs=2))
    small_pool = ctx.enter_context(tc.tile_pool(name="small", bufs=4))
    psum_tp = ctx.enter_context(tc.tile_pool(name="ptp", bufs=2, space="PSUM"))
    psum_mm = ctx.enter_context(tc.tile_pool(name="pmm", bufs=2, space="PSUM"))
    psum_sc = ctx.enter_context(tc.tile_pool(name="psc", bufs=2, space="PSUM"))
    psum_den = ctx.enter_context(tc.tile_pool(name="pden", bufs=2, space="PSUM"))

    # --- constants -----------------------------------------------------
    # gpsimd/iota writes are f32; launder through a DVE copy into f32r so
    # the BIR verifier sees a rounding producer for every matmul operand.
    ident_f = const_pool.tile([P, P], f32)
    make_identity(nc, ident_f)
    ident = const_pool.tile([P, P], f32r)
    nc.vector.tensor_copy(ident, ident_f)

    ones_f = const_pool.tile([P, 2], f32)
    nc.gpsimd.memset(ones_f, 1.0)
    ones_col = const_pool.tile([P, 2], f32r)
    nc.vector.tensor_copy(ones_col, ones_f)
    ones_row_f = const_pool.tile([1, P], f32)
    nc.gpsimd.memset(ones_row_f, 1.0)
    ones_row = const_pool.tile([1, P], f32r)
    nc.vector.tensor_copy(ones_row, ones_row_f)

    # Weight/bias loads are emitted lazily (after the first audio-half DMA)
    # so the first transposable input data leads the serial DMA queue; this
    # removes a ~12us PE startup stall waiting behind 6.5MB of constants.
    consts = {}

    def _load_consts():
        # small tensors first (bias ACT + bv broadcast gate PSUM drains),
        # then weights in first-use order; wq is emitted separately after
        # the x block-0 prefetch.
        bv_row = const_pool.tile([1, D], f32r)
        nc.sync.dma_start(bv_row, bv[None, :])
        bq_sb = const_pool.tile([P, CD], f32)
        nc.sync.dma_start(bq_sb, bq.rearrange("(c p) -> p c", p=P))
        bk_sb = const_pool.tile([P, CD], f32)
        nc.sync.dma_start(bk_sb, bk.rearrange("(c p) -> p c", p=P))
        wk_sb = const_pool.tile([P, CD, D], f32r)
        nc.sync.dma_start(wk_sb, wk.rearrange("(c p) n -> p c n", p=P))
        wv_sb = const_pool.tile([P, CD, D], f32r)
        nc.sync.dma_start(wv_sb, wv.rearrange("(c p) n -> p c n", p=P))
        # bv broadcast to all 128 partitions via a K=1 outer-product matmul
        bv_ps = psum_mm.tile([P, D], f32, tag="mm")
        nc.tensor.matmul(bv_ps, ones_row, bv_row, start=True, stop=True)
        bv_bc = const_pool.tile([P, D], f32)
        nc.vector.tensor_copy(bv_bc, bv_ps)
        consts.update(wk_sb=wk_sb, wv_sb=wv_sb,
                      bq_sb=bq_sb, bk_sb=bk_sb, bv_bc=bv_bc)

    def _load_wq():
        wq_sb = const_pool.tile([P, CD, D], f32r)
        nc.sync.dma_start(wq_sb, wq.rearrange("(c p) n -> p c n", p=P))
        consts.update(wq_sb=wq_sb)

    x_pre = {}
    for b in range(BL):
        # --- per-batch: audio transpose, K^T, V; one audio half at a time
        # so half-0 compute never waits behind the half-1 DMA -----------
        aT = batch_pool.tile([P, CD, S], f32r, tag="aT")
        kT = batch_pool.tile([P, CD, S], f32r, tag="kT")
        v_sb = batch_pool.tile([P, SC, D], f32r, tag="v")
        for half in range(2):
            a_half = work_pool.tile([P, CD, D], f32r, tag="x")
            nc.sync.dma_start(
                a_half, audio[b].rearrange("(t c p) n -> t p c n", p=P, c=CD)[half]
            )
            if b == 0 and half == 0:
                _load_consts()
                x_pre0 = work_pool.tile([P, CD, D], f32r, tag="x", name="x_pre0")
                nc.sync.dma_start(
                    x_pre0, x[0].rearrange("(t c p) n -> t p c n", p=P, c=CD)[0]
                )
                x_pre[(0, 0)] = x_pre0
                _load_wq()
            for dc in range(CD):
                tp_ps = psum_tp.tile([P, HWB], f32r, tag="tp")
                for c in range(CD):
                    nc.tensor.matmul(
                        tp_ps[:, c * P : (c + 1) * P],
                        a_half[:, c, dc * P : (dc + 1) * P],
                        ident,
                        is_transpose=True,
                    )
                nc.vector.tensor_copy(aT[:, dc, half * 512 : (half + 1) * 512], tp_ps)

            for m in range(CD):
                mm_ps = psum_mm.tile([P, 512], f32, tag="mm")
                for c in range(CD):
                    nc.tensor.matmul(
                        mm_ps,
                        consts["wk_sb"][:, c, m * P : (m + 1) * P],
                        aT[:, c, half * 512 : (half + 1) * 512],
                        start=(c == 0),
                        stop=(c == CD - 1),
                    )
                nc.scalar.activation(
                    kT[:, m, half * 512 : (half + 1) * 512],
                    mm_ps,
                    AFT.Identity,
                    bias=consts["bk_sb"][:, m, None],
                    scale=1.0,
                )

            for g in range(half * 4, half * 4 + 4):
                mm_ps = psum_mm.tile([P, D], f32, tag="mm")
                for c in range(CD):
                    nc.tensor.matmul(
                        mm_ps,
                        aT[:, c, g * P : (g + 1) * P],
                        consts["wv_sb"][:, c, :],
                        start=(c == 0),
                        stop=(c == CD - 1),
                    )
                nc.vector.tensor_add(v_sb[:, g, :], mm_ps, consts["bv_bc"])

        # --- hw blocks -------------------------------------------------
        for blk in range(NBLK):
            x_sb = x_pre.pop((b, blk), None)
            if x_sb is None:
                x_sb = work_pool.tile([P, CD, D], f32r, tag="x")
                nc.sync.dma_start(
                    x_sb, x[b].rearrange("(t c p) n -> t p c n", p=P, c=CD)[blk]
                )

            xT = work_pool.tile([P, CD, HWB], f32r, tag="xT")
            for dc in range(CD):
                tp_ps = psum_tp.tile([P, HWB], f32r, tag="tp")
                for c in range(CD):
                    nc.tensor.matmul(
                        tp_ps[:, c * P : (c + 1) * P],
                        x_sb[:, c, dc * P : (dc + 1) * P],
                        ident,
                        is_transpose=True,
                    )
                nc.vector.tensor_copy(xT[:, dc, :], tp_ps)

            qT = work_pool.tile([P, CD, HWB], f32r, tag="qT")
            for m in range(CD):
                mm_ps = psum_mm.tile([P, HWB], f32, tag="mm")
                for c in range(CD):
                    nc.tensor.matmul(
                        mm_ps,
                        consts["wq_sb"][:, c, m * P : (m + 1) * P],
                        xT[:, c, :],
                        start=(c == 0),
                        stop=(c == CD - 1),
                    )
                nc.scalar.activation(
                    qT[:, m, :], mm_ps, AFT.Identity, bias=consts["bq_sb"][:, m, None], scale=1.0
                )

            ex = work_pool.tile([P, SC, HWB], f32r, tag="ex")
            for g in range(SC):
                sc_ps = psum_sc.tile([P, HWB], f32, tag="sc")
                for m in range(CD):
                    nc.tensor.matmul(
                        sc_ps,
                        kT[:, m, g * P : (g + 1) * P],
                        qT[:, m, :],
                        start=(m == 0),
                        stop=(m == CD - 1),
                    )
                nc.scalar.activation(
                    ex[:, g, :], sc_ps, AFT.Exp, bias=0.0, scale=SCALE
                )

            out_sb = work_pool.tile([P, CD, D], f32, tag="o")
            for h in range(CD):
                o_ps = psum_mm.tile([P, D], f32, tag="mm")
                d_ps = psum_den.tile([P, 2], f32, tag="den")
                for g in range(SC):
                    lhs = ex[:, g, h * P : (h + 1) * P]
                    nc.tensor.matmul(
                        o_ps, lhs, v_sb[:, g, :], start=(g == 0), stop=(g == SC - 1)
                    )
                    nc.tensor.matmul(
                        d_ps, lhs, ones_col, start=(g == 0), stop=(g == SC - 1)
                    )
                rec = small_pool.tile([P, 1], f32, tag="rec")
                nc.vector.reciprocal(rec, d_ps[:, 0:1])
                nc.scalar.activation(
                    out_sb[:, h, :], o_ps, AFT.Copy, bias=0.0, scale=rec
                )
            nc.sync.dma_start(
                out[b].rearrange("(t h p) n -> t p h n", p=P, h=CD)[blk], out_sb
            )


_NC_CACHE = None


def _get_nc():
    global _NC_CACHE
    if _NC_CACHE is None:
        _NC_CACHE = build_nc()
    return _NC_CACHE


def kernel(**inputs):
    x = np.ascontiguousarray(np.asarray(inputs["x"], dtype=np.float32))
    audio = np.ascontiguousarray(np.asarray(inputs["audio_embed"], dtype=np.float32))
    wq = np.ascontiguousarray(np.asarray(inputs["Wq"], dtype=np.float32))
    bq = np.ascontiguousarray(np.asarray(inputs["bq"], dtype=np.float32))
    wk = np.ascontiguousarray(np.asarray(inputs["Wk"], dtype=np.float32))
    bk = np.ascontiguousarray(np.asarray(inputs["bk"], dtype=np.float32))
    wv = np.ascontiguousarray(np.asarray(inputs["Wv"], dtype=np.float32))
    bv = np.ascontiguousarray(np.asarray(inputs["bv"], dtype=np.float32))

    nc = _get_nc()
    in_maps = []
    for i in range(N_CORES):
        in_maps.append(
            {
                "x": np.ascontiguousarray(x[i * BL : (i + 1) * BL]),
                "audio_embed": np.ascontiguousarray(audio[i * BL : (i + 1) * BL]),
                "Wq": wq,
                "bq": bq,
                "Wk": wk,
                "bk": bk,
                "Wv": wv,
                "bv": bv,
            }
        )
    res = run_bass_kernel_spmd(nc, in_maps, core_ids=list(range(N_CORES)))
    return np.concatenate([res.results[i]["out"] for i in range(N_CORES)], axis=0)



# revision 2
# speedup vs baseline: 1.5306x; 1.5306x over previous
"""Cross-attention block kernel for Trainium2 (8 NeuronCores, data-parallel).

Computes, for full inputs:
    Q = x @ Wq + bq            [B, HW, D]
    K = a @ Wk + bk            [B, S, D]
    V = a @ Wv + bv            [B, S, D]
    out = softmax(Q K^T / sqrt(D)) @ V

Sharding: batch (B=16) split across 8 cores, 2 batches per core. Weights
replicated. No collectives needed.

Per-core kernel strategy:
  - All inputs host-cast to bf16 (halves DMA, transposes run 1.0 cyc/row,
    projection matmuls run at full PE rate like fp32r).
  - PE-transpose x and audio tiles into d-major layouts (xT, aT) in bf16.
  - qT8 = fp8e4(Wq^T @ xT + bq), kT8 = fp8e4(Wk^T @ aT + bk): ACT applies
    the bias straight out of PSUM and emits fp8e4.
  - v8 = fp8e4(aT^T @ Wv + bv) via DVE add out of PSUM.
  - scoresT[s, hw] = kT8^T @ qT8 with fp8 DoubleRow matmuls (2 k-tiles per
    instruction, 0.5 cycles/row): 4x the fp32r rate. ACT computes
    ex = exp(scale * scoresT) in bf16 (scores std ~0.33, exp is safe).
  - Softmax numerator uses mean-centering to keep fp8 quantization error
    small: weights ex = 1 + u with u = ex - 1 (|u| ~ 0.36), so
      out_num = colV + u8 @ V8,  colV[d] = sum_s V[s, d]  (exact, f32)
    u8 = fp8e4(ex - 1) carries quantization noise on u (~0.36), not on the
    full weight (~1.0): ~3x less error. colV is accumulated into the same
    PSUM bank via a K=1 f32r matmul (ones x colV row), computed per batch
    as colA @ Wv + S*bv with colA = rowsum(aT) (exact in f32).
  - Denominator den = S + sum_s u8 via a DoubleRow matmul against a ones
    column; DVE adds S, takes the reciprocal, and scales the output.
  - Output written as bf16 (harness tolerance 2e-2; bf16 adds ~0.2%).

fp8 DoubleRow error budget (measured ~1e-2 level): score quantization
(Q,K in fp8e4) ~1.1%, u8 ~0.8%, V8 (enters only the u-weighted term) ~0.8%.
"""

from contextlib import ExitStack

import numpy as np
import ml_dtypes

import concourse.bass as bass
import concourse.bacc as bacc
import concourse.mybir as mybir
import concourse.tile as tile
from concourse.bass_utils import run_bass_kernel_spmd
from concourse.masks import make_identity

P = 128
D = 512          # d_query == d_audio == d_out
CD = D // P      # 4 chunks of the feature dim
HW = 4096        # queries per batch
S = 1024         # keys per batch
SC = S // P      # 8 s-chunks
HWB = 512        # hw rows processed per block
NBLK = HW // HWB
B_FULL = 16
N_CORES = 8
BL = B_FULL // N_CORES  # 2 batches per core
SCALE = 1.0 / float(np.sqrt(D))

f32 = mybir.dt.float32
f32r = mybir.dt.float32r
bf16 = mybir.dt.bfloat16
fp8 = mybir.dt.float8e4
AFT = mybir.ActivationFunctionType
DR = mybir.MatmulPerfMode.DoubleRow

bf16_np = ml_dtypes.bfloat16


def build_nc():
    nc = bacc.Bacc("TRN2", target_bir_lowering=False, debug=False)

    x = nc.dram_tensor("x", [BL, HW, D], bf16, kind="ExternalInput").ap()
    audio = nc.dram_tensor("audio_embed", [BL, S, D], bf16, kind="ExternalInput").ap()
    wq = nc.dram_tensor("Wq", [D, D], bf16, kind="ExternalInput").ap()
    bq = nc.dram_tensor("bq", [D], f32, kind="ExternalInput").ap()
    wk = nc.dram_tensor("Wk", [D, D], bf16, kind="ExternalInput").ap()
    bk = nc.dram_tensor("bk", [D], f32, kind="ExternalInput").ap()
    wv = nc.dram_tensor("Wv", [D, D], bf16, kind="ExternalInput").ap()
    bv = nc.dram_tensor("bv", [D], f32, kind="ExternalInput").ap()
    out = nc.dram_tensor("out", [BL, HW, D], bf16, kind="ExternalOutput").ap()

    with tile.TileContext(nc) as tc:
        with ExitStack() as ctx:
            _body(ctx, tc, x, audio, wq, bq, wk, bk, wv, bv, out)

    nc.compile()
    return nc


def _body(ctx, tc, x, audio, wq, bq, wk, bk, wv, bv, out):
    nc = tc.nc

    const_pool = ctx.enter_context(tc.tile_pool(name="const", bufs=1))
    batch_pool = ctx.enter_context(tc.tile_pool(name="batch", bufs=2))
    work_pool = ctx.enter_context(tc.tile_pool(name="work", bufs=2))
    small_pool = ctx.enter_context(tc.tile_pool(name="small", bufs=4))
    psum_tp = ctx.enter_context(tc.tile_pool(name="ptp", bufs=1, space="PSUM"))
    psum_mm = ctx.enter_context(tc.tile_pool(name="pmm", bufs=2, space="PSUM"))
    psum_sc = ctx.enter_context(tc.tile_pool(name="psc", bufs=2, space="PSUM"))
    psum_o = ctx.enter_context(tc.tile_pool(name="po", bufs=2, space="PSUM"))
    psum_den = ctx.enter_context(tc.tile_pool(name="pden", bufs=1, space="PSUM"))

    # --- constants -----------------------------------------------------
    ident_f = const_pool.tile([P, P], f32)
    make_identity(nc, ident_f)
    ident = const_pool.tile([P, P], bf16)
    nc.vector.tensor_copy(ident, ident_f)

    # ones for the den DoubleRow matmul: rhs [K=128, 2 k-tiles, 2 cols]
    ones22_f = const_pool.tile([P, 2, 2], f32)
    nc.gpsimd.memset(ones22_f, 1.0)
    ones22_8 = const_pool.tile([P, 2, 2], fp8)
    nc.vector.tensor_copy(ones22_8, ones22_f)
    # ones row for K=1 broadcast matmuls (f32r chain)
    ones_row_f = const_pool.tile([1, P], f32)
    nc.gpsimd.memset(ones_row_f, 1.0)
    ones_row = const_pool.tile([1, P], f32r)
    nc.vector.tensor_copy(ones_row, ones_row_f)

    # Weight/bias loads are emitted lazily (after the first audio-half DMA)
    # so the first transposable input data leads the serial DMA queue.
    consts = {}

    def _load_consts():
        bv_row = const_pool.tile([1, D], f32)
        nc.sync.dma_start(bv_row, bv[None, :])
        bq_sb = const_pool.tile([P, CD], f32)
        nc.sync.dma_start(bq_sb, bq.rearrange("(c p) -> p c", p=P))
        bk_sb = const_pool.tile([P, CD], f32)
        nc.sync.dma_start(bk_sb, bk.rearrange("(c p) -> p c", p=P))
        wk_sb = const_pool.tile([P, CD, D], bf16)
        nc.sync.dma_start(wk_sb, wk.rearrange("(c p) n -> p c n", p=P))
        wv_sb = const_pool.tile([P, CD, D], bf16)
        nc.sync.dma_start(wv_sb, wv.rearrange("(c p) n -> p c n", p=P))
        # bv * S for the colV row (colV = colA @ Wv + S*bv)
        bv1024 = const_pool.tile([1, D], f32)
        nc.vector.tensor_scalar_mul(bv1024, bv_row, float(S))
        # bv broadcast to all 128 partitions via a K=1 outer-product matmul
        bv_row_r = const_pool.tile([1, D], f32r)
        nc.vector.tensor_copy(bv_row_r, bv_row)
        bv_ps = psum_mm.tile([P, D], f32, tag="mm")
        nc.tensor.matmul(bv_ps, ones_row, bv_row_r, start=True, stop=True)
        bv_bc = const_pool.tile([P, D], f32)
        nc.vector.tensor_copy(bv_bc, bv_ps)
        consts.update(wk_sb=wk_sb, wv_sb=wv_sb,
                      bq_sb=bq_sb, bk_sb=bk_sb, bv_bc=bv_bc, bv1024=bv1024)

    def _load_wq():
        wq_sb = const_pool.tile([P, CD, D], bf16)
        nc.sync.dma_start(wq_sb, wq.rearrange("(c p) n -> p c n", p=P))
        consts.update(wq_sb=wq_sb)

    x_pre = {}
    for b in range(BL):
        # --- per-batch: audio transpose, K^T (fp8), V (fp8), colV ------
        aT = batch_pool.tile([P, CD, S], bf16, tag="aT")
        kT8 = batch_pool.tile([P, CD, S], fp8, tag="kT")
        v8 = batch_pool.tile([P, SC, D], fp8, tag="v")
        for half in range(2):
            a_half = work_pool.tile([P, CD, D], bf16, tag="x")
            nc.sync.dma_start(
                a_half, audio[b].rearrange("(t c p) n -> t p c n", p=P, c=CD)[half]
            )
            if b == 0 and half == 0:
                _load_consts()
                x_pre0 = work_pool.tile([P, CD, D], bf16, tag="x", name="x_pre0")
                nc.sync.dma_start(
                    x_pre0, x[0].rearrange("(t c p) n -> t p c n", p=P, c=CD)[0]
                )
                x_pre[(0, 0)] = x_pre0
                _load_wq()
            for dc in range(CD):
                tp_ps = psum_tp.tile([P, HWB], bf16, tag="tp")
                for c in range(CD):
                    nc.tensor.matmul(
                        tp_ps[:, c * P : (c + 1) * P],
                        a_half[:, c, dc * P : (dc + 1) * P],
                        ident,
                        is_transpose=True,
                    )
                nc.vector.tensor_copy(aT[:, dc, half * 512 : (half + 1) * 512], tp_ps)

            for m in range(CD):
                mm_ps = psum_mm.tile([P, 512], f32, tag="mm")
                for c in range(CD):
                    nc.tensor.matmul(
                        mm_ps,
                        consts["wk_sb"][:, c, m * P : (m + 1) * P],
                        aT[:, c, half * 512 : (half + 1) * 512],
                        start=(c == 0),
                        stop=(c == CD - 1),
                    )
                nc.scalar.activation(
                    kT8[:, m, half * 512 : (half + 1) * 512],
                    mm_ps,
                    AFT.Identity,
                    bias=consts["bk_sb"][:, m, None],
                    scale=1.0,
                )

            for g in range(half * 4, half * 4 + 4):
                mm_ps = psum_mm.tile([P, D], f32, tag="mm")
                for c in range(CD):
                    nc.tensor.matmul(
                        mm_ps,
                        aT[:, c, g * P : (g + 1) * P],
                        consts["wv_sb"][:, c, :],
                        start=(c == 0),
                        stop=(c == CD - 1),
                    )
                nc.vector.tensor_add(v8[:, g, :], mm_ps, consts["bv_bc"])

        # colV[d] = sum_s V[s, d] = colA @ Wv + S*bv (exact, f32 path)
        colA_f = small_pool.tile([P, CD], f32, tag="colA")
        nc.vector.tensor_reduce(
            colA_f, aT, axis=mybir.AxisListType.X, op=mybir.AluOpType.add
        )
        colA_b = small_pool.tile([P, CD], bf16, tag="colAb")
        nc.vector.tensor_copy(colA_b, colA_f)
        cv_ps = psum_sc.tile([P, HWB], f32, tag="sc")
        for c in range(CD):
            nc.tensor.matmul(
                cv_ps[0:1, :],
                colA_b[:, c, None],
                consts["wv_sb"][:, c, :],
                start=(c == 0),
                stop=(c == CD - 1),
            )
        colV_r = batch_pool.tile([1, D], f32r, tag="colV")
        nc.vector.tensor_add(colV_r, cv_ps[0:1, :], consts["bv1024"])

        # --- hw blocks -------------------------------------------------
        for blk in range(NBLK):
            x_sb = x_pre.pop((b, blk), None)
            if x_sb is None:
                x_sb = work_pool.tile([P, CD, D], bf16, tag="x")
                nc.sync.dma_start(
                    x_sb, x[b].rearrange("(t c p) n -> t p c n", p=P, c=CD)[blk]
                )

            xT = work_pool.tile([P, CD, HWB], bf16, tag="xT")
            for dc in range(CD):
                tp_ps = psum_tp.tile([P, HWB], bf16, tag="tp")
                for c in range(CD):
                    nc.tensor.matmul(
                        tp_ps[:, c * P : (c + 1) * P],
                        x_sb[:, c, dc * P : (dc + 1) * P],
                        ident,
                        is_transpose=True,
                    )
                nc.vector.tensor_copy(xT[:, dc, :], tp_ps)

            qT8 = work_pool.tile([P, CD, HWB], fp8, tag="qT")
            for m in range(CD):
                mm_ps = psum_mm.tile([P, HWB], f32, tag="mm")
                for c in range(CD):
                    nc.tensor.matmul(
                        mm_ps,
                        consts["wq_sb"][:, c, m * P : (m + 1) * P],
                        xT[:, c, :],
                        start=(c == 0),
                        stop=(c == CD - 1),
                    )
                nc.scalar.activation(
                    qT8[:, m, :], mm_ps, AFT.Identity,
                    bias=consts["bq_sb"][:, m, None], scale=1.0,
                )

            ex_bf = work_pool.tile([P, SC, HWB], bf16, tag="ex")
            for g in range(SC):
                sc_ps = psum_sc.tile([P, HWB], f32, tag="sc")
                nc.tensor.matmul(
                    sc_ps, kT8[:, 0:2, g * P : (g + 1) * P], qT8[:, 0:2, :],
                    start=True, stop=False, perf_mode=DR,
                )
                nc.tensor.matmul(
                    sc_ps, kT8[:, 2:4, g * P : (g + 1) * P], qT8[:, 2:4, :],
                    start=False, stop=True, perf_mode=DR,
                )
                nc.scalar.activation(
                    ex_bf[:, g, :], sc_ps, AFT.Exp, bias=0.0, scale=SCALE
                )

            # u8 = fp8(ex - 1); split across ACT and DVE for engine balance
            u8 = work_pool.tile([P, SC, HWB], fp8, tag="u8")
            nc.scalar.activation(
                u8[:, 0:4, :], ex_bf[:, 0:4, :], AFT.Copy, bias=-1.0, scale=1.0
            )
            nc.vector.tensor_scalar_add(u8[:, 4:8, :], ex_bf[:, 4:8, :], -1.0)

            out_sb = work_pool.tile([P, CD, D], bf16, tag="o")
            for h in range(CD):
                o_ps = psum_o.tile([P, D], f32, tag="o")
                d_ps = psum_den.tile([P, 2], f32, tag="den")
                # colV broadcast into PSUM opens the accumulation group
                nc.tensor.matmul(o_ps, ones_row, colV_r, start=True, stop=False)
                for i in range(CD):
                    lhs = u8[:, 2 * i : 2 * i + 2, h * P : (h + 1) * P]
                    nc.tensor.matmul(
                        o_ps, lhs, v8[:, 2 * i : 2 * i + 2, :],
                        start=False, stop=(i == CD - 1), perf_mode=DR,
                    )
                    nc.tensor.matmul(
                        d_ps, lhs, ones22_8,
                        start=(i == 0), stop=(i == CD - 1), perf_mode=DR,
                    )
                den_t = small_pool.tile([P, 1], f32, tag="den")
                nc.vector.tensor_scalar_add(den_t, d_ps[:, 0:1], float(S))
                rec = small_pool.tile([P, 1], f32, tag="rec")
                nc.vector.reciprocal(rec, den_t)
                nc.vector.tensor_scalar_mul(out_sb[:, h, :], o_ps, rec[:, 0:1])
            nc.sync.dma_start(
                out[b].rearrange("(t h p) n -> t p h n", p=P, h=CD)[blk], out_sb
            )


_NC_CACHE = None


def _get_nc():
    global _NC_CACHE
    if _NC_CACHE is None:
        _NC_CACHE = build_nc()
    return _NC_CACHE


def kernel(**inputs):
    x = np.asarray(inputs["x"], dtype=np.float32).astype(bf16_np)
    audio = np.asarray(inputs["audio_embed"], dtype=np.float32).astype(bf16_np)
    wq = np.asarray(inputs["Wq"], dtype=np.float32).astype(bf16_np)
    bq = np.ascontiguousarray(np.asarray(inputs["bq"], dtype=np.float32))
    wk = np.asarray(inputs["Wk"], dtype=np.float32).astype(bf16_np)
    bk = np.ascontiguousarray(np.asarray(inputs["bk"], dtype=np.float32))
    wv = np.asarray(inputs["Wv"], dtype=np.float32).astype(bf16_np)
    bv = np.ascontiguousarray(np.asarray(inputs["bv"], dtype=np.float32))

    nc = _get_nc()
    in_maps = []
    for i in range(N_CORES):
        in_maps.append(
            {
                "x": np.ascontiguousarray(x[i * BL : (i + 1) * BL]),
                "audio_embed": np.ascontiguousarray(audio[i * BL : (i + 1) * BL]),
                "Wq": wq,
                "bq": bq,
                "Wk": wk,
                "bk": bk,
                "Wv": wv,
                "bv": bv,
            }
        )
    res = run_bass_kernel_spmd(nc, in_maps, core_ids=list(range(N_CORES)))
    return np.concatenate(
        [res.results[i]["out"].astype(np.float32) for i in range(N_CORES)], axis=0
    )


# revision 3
# speedup vs baseline: 1.6467x; 1.0758x over previous
"""Cross-attention block kernel for Trainium2 (8 NeuronCores, data-parallel).

Computes, for full inputs:
    Q = x @ Wq + bq            [B, HW, D]
    K = a @ Wk + bk            [B, S, D]
    V = a @ Wv + bv            [B, S, D]
    out = softmax(Q K^T / sqrt(D)) @ V

Sharding: batch (B=16) split across 8 cores, 2 batches per core. Weights
replicated. No collectives needed.

Per-core kernel strategy:
  - All inputs host-cast to bf16 (halves DMA, transposes run 1.0 cyc/row,
    projection matmuls run at full PE rate like fp32r).
  - PE-transpose x and audio tiles into d-major layouts (xT, aT) in bf16.
  - qT8 = fp8e4(Wq^T @ xT + bq), kT8 = fp8e4(Wk^T @ aT + bk): ACT applies
    the bias straight out of PSUM and emits fp8e4.
  - v8 = fp8e4(aT^T @ Wv + bv) via DVE add out of PSUM.
  - scoresT[s, hw] = kT8^T @ qT8 with fp8 DoubleRow matmuls (2 k-tiles per
    instruction, 0.5 cycles/row): 4x the fp32r rate. ACT computes
    ex = exp(scale * scoresT) in bf16 (scores std ~0.33, exp is safe).
  - Softmax numerator uses mean-centering to keep fp8 quantization error
    small: weights ex = 1 + u with u = ex - 1 (|u| ~ 0.36), so
      out_num = colV + u8 @ V8,  colV[d] = sum_s V[s, d]  (exact-ish)
    u8 = fp8e4(ex - 1) carries quantization noise on u (~0.36), not on the
    full weight (~1.0): ~3x less error. colV is computed per batch as
    colA @ Wv + S*bv with colA = rowsum(aT), split into an fp8 hi+lo pair
    (residual splitting keeps the error ~0.06%) and accumulated into the
    output PSUM group via a single K=1 DoubleRow matmul.
  - Denominator den = S + sum_s u8 via a DoubleRow matmul against a ones
    column; DVE adds S, takes the reciprocal, and scales the output.
  - Output written as bf16 (harness tolerance 2e-2; bf16 adds ~0.2%).

The emission order is software-pipelined: the output phase (attn @ V) of
block i-1 is emitted after the compute phase of block i, so the PE never
waits for the same block's exp/u8 chain; x tiles are prefetched one block
ahead and audio one batch ahead.

fp8 error budget (measured): scores (Q,K in fp8e4) 1.18e-2, u8 0.63e-2,
V8 0.62e-2, bf16 inputs 0.27e-2 -> total ~1.46e-2 vs the 2e-2 gate.
"""

from contextlib import ExitStack

import numpy as np
import ml_dtypes

import concourse.bass as bass
import concourse.bacc as bacc
import concourse.mybir as mybir
import concourse.tile as tile
from concourse.bass_utils import run_bass_kernel_spmd
from concourse.masks import make_identity

P = 128
D = 512          # d_query == d_audio == d_out
CD = D // P      # 4 chunks of the feature dim
HW = 4096        # queries per batch
S = 1024         # keys per batch
SC = S // P      # 8 s-chunks
HWB = 512        # hw rows processed per block
NBLK = HW // HWB
B_FULL = 16
N_CORES = 8
BL = B_FULL // N_CORES  # 2 batches per core
SCALE = 1.0 / float(np.sqrt(D))

f32 = mybir.dt.float32
f32r = mybir.dt.float32r
bf16 = mybir.dt.bfloat16
fp8 = mybir.dt.float8e4
AFT = mybir.ActivationFunctionType
ALU = mybir.AluOpType
DR = mybir.MatmulPerfMode.DoubleRow

bf16_np = ml_dtypes.bfloat16


def build_nc():
    nc = bacc.Bacc("TRN2", target_bir_lowering=False, debug=False)

    x = nc.dram_tensor("x", [BL, HW, D], bf16, kind="ExternalInput").ap()
    audio = nc.dram_tensor("audio_embed", [BL, S, D], bf16, kind="ExternalInput").ap()
    wq = nc.dram_tensor("Wq", [D, D], bf16, kind="ExternalInput").ap()
    bq = nc.dram_tensor("bq", [D], f32, kind="ExternalInput").ap()
    wk = nc.dram_tensor("Wk", [D, D], bf16, kind="ExternalInput").ap()
    bk = nc.dram_tensor("bk", [D], f32, kind="ExternalInput").ap()
    wv = nc.dram_tensor("Wv", [D, D], bf16, kind="ExternalInput").ap()
    bv = nc.dram_tensor("bv", [D], f32, kind="ExternalInput").ap()
    out = nc.dram_tensor("out", [BL, HW, D], bf16, kind="ExternalOutput").ap()

    with tile.TileContext(nc) as tc:
        with ExitStack() as ctx:
            _body(ctx, tc, x, audio, wq, bq, wk, bk, wv, bv, out)

    nc.compile()
    return nc


def _body(ctx, tc, x, audio, wq, bq, wk, bk, wv, bv, out):
    nc = tc.nc

    const_pool = ctx.enter_context(tc.tile_pool(name="const", bufs=1))
    batch_pool = ctx.enter_context(tc.tile_pool(name="batch", bufs=2))
    work_pool = ctx.enter_context(tc.tile_pool(name="work", bufs=2))
    small_pool = ctx.enter_context(tc.tile_pool(name="small", bufs=4))
    psum_tp = ctx.enter_context(tc.tile_pool(name="ptp", bufs=1, space="PSUM"))
    psum_mm = ctx.enter_context(tc.tile_pool(name="pmm", bufs=2, space="PSUM"))
    psum_sc = ctx.enter_context(tc.tile_pool(name="psc", bufs=2, space="PSUM"))
    psum_o = ctx.enter_context(tc.tile_pool(name="po", bufs=2, space="PSUM"))
    psum_den = ctx.enter_context(tc.tile_pool(name="pden", bufs=1, space="PSUM"))

    # --- constants -----------------------------------------------------
    ident_f = const_pool.tile([P, P], f32)
    make_identity(nc, ident_f)
    ident = const_pool.tile([P, P], bf16)
    nc.vector.tensor_copy(ident, ident_f)

    # ones for the den DoubleRow matmul: rhs [K=128, 2 k-tiles, 2 cols]
    ones22_f = const_pool.tile([P, 2, 2], f32)
    nc.gpsimd.memset(ones22_f, 1.0)
    ones22_8 = const_pool.tile([P, 2, 2], fp8)
    nc.vector.tensor_copy(ones22_8, ones22_f)
    # ones lhsT for the K=1 colV DoubleRow matmul: [1, 2 k-tiles, 128]
    ones12_f = const_pool.tile([1, 2, P], f32)
    nc.gpsimd.memset(ones12_f, 1.0)
    ones12_8 = const_pool.tile([1, 2, P], fp8)
    nc.vector.tensor_copy(ones12_8, ones12_f)
    # ones row for the K=1 bv broadcast matmul (f32r chain)
    ones_row_f = const_pool.tile([1, P], f32)
    nc.gpsimd.memset(ones_row_f, 1.0)
    ones_row = const_pool.tile([1, P], f32r)
    nc.vector.tensor_copy(ones_row, ones_row_f)

    # Weight/bias loads are emitted lazily (after the first audio-half DMA)
    # so the first transposable input data leads the serial DMA queue.
    consts = {}

    def _load_consts():
        bv_row = const_pool.tile([1, D], f32)
        nc.sync.dma_start(bv_row, bv[None, :])
        bq_sb = const_pool.tile([P, CD], f32)
        nc.sync.dma_start(bq_sb, bq.rearrange("(c p) -> p c", p=P))
        bk_sb = const_pool.tile([P, CD], f32)
        nc.sync.dma_start(bk_sb, bk.rearrange("(c p) -> p c", p=P))
        wk_sb = const_pool.tile([P, CD, D], bf16)
        nc.sync.dma_start(wk_sb, wk.rearrange("(c p) n -> p c n", p=P))
        wv_sb = const_pool.tile([P, CD, D], bf16)
        nc.sync.dma_start(wv_sb, wv.rearrange("(c p) n -> p c n", p=P))
        # bv * S for the colV row (colV = colA @ Wv + S*bv)
        bv1024 = const_pool.tile([1, D], f32)
        nc.vector.tensor_scalar_mul(bv1024, bv_row, float(S))
        # bv broadcast to all 128 partitions via a K=1 outer-product matmul
        bv_row_r = const_pool.tile([1, D], f32r)
        nc.vector.tensor_copy(bv_row_r, bv_row)
        bv_ps = psum_mm.tile([P, D], f32, tag="mm")
        nc.tensor.matmul(bv_ps, ones_row, bv_row_r, start=True, stop=True)
        bv_bc = const_pool.tile([P, D], f32)
        nc.vector.tensor_copy(bv_bc, bv_ps)
        consts.update(wk_sb=wk_sb, wv_sb=wv_sb,
                      bq_sb=bq_sb, bk_sb=bk_sb, bv_bc=bv_bc, bv1024=bv1024)

    def _load_wq():
        wq_sb = const_pool.tile([P, CD, D], bf16)
        nc.sync.dma_start(wq_sb, wq.rearrange("(c p) n -> p c n", p=P))
        consts.update(wq_sb=wq_sb)

    def _dma_x(b, blk):
        t = work_pool.tile([P, CD, D], bf16, tag="x")
        nc.sync.dma_start(
            t, x[b].rearrange("(t c p) n -> t p c n", p=P, c=CD)[blk]
        )
        return t

    def _dma_audio_half(b, half):
        t = work_pool.tile([P, CD, D], bf16, tag="ah")
        nc.sync.dma_start(
            t, audio[b].rearrange("(t c p) n -> t p c n", p=P, c=CD)[half]
        )
        return t

    x_pre = {}
    audio_pre = {}

    def _prep_batch(b):
        """Audio transpose, K^T (fp8), V (fp8), colV pair for batch b."""
        aT = batch_pool.tile([P, CD, S], bf16, tag="aT")
        kT8 = batch_pool.tile([P, CD, S], fp8, tag="kT")
        v8 = batch_pool.tile([P, SC, D], fp8, tag="v")
        for half in range(2):
            a_half = audio_pre.pop((b, half), None)
            if a_half is None:
                a_half = _dma_audio_half(b, half)
            if b == 0 and half == 0:
                _load_consts()
                x_pre[(0, 0)] = _dma_x(0, 0)
                _load_wq()
            for dc in range(CD):
                tp_ps = psum_tp.tile([P, HWB], bf16, tag="tp")
                for c in range(CD):
                    nc.tensor.matmul(
                        tp_ps[:, c * P : (c + 1) * P],
                        a_half[:, c, dc * P : (dc + 1) * P],
                        ident,
                        is_transpose=True,
                    )
                nc.vector.tensor_copy(aT[:, dc, half * 512 : (half + 1) * 512], tp_ps)

            for m in range(CD):
                mm_ps = psum_mm.tile([P, 512], f32, tag="mm")
                for c in range(CD):
                    nc.tensor.matmul(
                        mm_ps,
                        consts["wk_sb"][:, c, m * P : (m + 1) * P],
                        aT[:, c, half * 512 : (half + 1) * 512],
                        start=(c == 0),
                        stop=(c == CD - 1),
                    )
                nc.scalar.activation(
                    kT8[:, m, half * 512 : (half + 1) * 512],
                    mm_ps,
                    AFT.Identity,
                    bias=consts["bk_sb"][:, m, None],
                    scale=1.0,
                )

            for g in range(half * 4, half * 4 + 4):
                mm_ps = psum_mm.tile([P, D], f32, tag="mm")
                for c in range(CD):
                    nc.tensor.matmul(
                        mm_ps,
                        aT[:, c, g * P : (g + 1) * P],
                        consts["wv_sb"][:, c, :],
                        start=(c == 0),
                        stop=(c == CD - 1),
                    )
                nc.vector.tensor_add(v8[:, g, :], mm_ps, consts["bv_bc"])

        # colV[d] = sum_s V[s, d] = colA @ Wv + S*bv, split to fp8 hi+lo
        colA_f = small_pool.tile([P, CD], f32, tag="colA")
        nc.vector.tensor_reduce(
            colA_f, aT, axis=mybir.AxisListType.X, op=ALU.add
        )
        colA_b = small_pool.tile([P, CD], bf16, tag="colAb")
        nc.vector.tensor_copy(colA_b, colA_f)
        cv_ps = psum_sc.tile([P, HWB], f32, tag="sc")
        for c in range(CD):
            nc.tensor.matmul(
                cv_ps[0:1, :],
                colA_b[:, c, None],
                consts["wv_sb"][:, c, :],
                start=(c == 0),
                stop=(c == CD - 1),
            )
        colV_f = small_pool.tile([1, D], f32, tag="colVf")
        nc.vector.tensor_add(colV_f, cv_ps[0:1, :], consts["bv1024"])
        colV2 = batch_pool.tile([1, 2, D], fp8, tag="colV")
        nc.vector.tensor_copy(colV2[:, 0, :], colV_f)
        nc.vector.tensor_sub(colV2[:, 1, :], colV_f, colV2[:, 0, :])
        return dict(aT=aT, kT8=kT8, v8=v8, colV2=colV2)

    def _block_A(b, blk, bt):
        """Transposes, Q^T (fp8), scores, exp, u8 for block (b, blk)."""
        # prefetch the next x block (or next batch's audio) while computing
        nxt = (b, blk + 1) if blk + 1 < NBLK else (b + 1, 0)
        if nxt[0] < BL and nxt not in x_pre:
            x_pre[nxt] = _dma_x(*nxt)
        if blk == NBLK - 2 and b + 1 < BL:
            for half in range(2):
                audio_pre[(b + 1, half)] = _dma_audio_half(b + 1, half)

        x_sb = x_pre.pop((b, blk))

        xT = work_pool.tile([P, CD, HWB], bf16, tag="xT")
        for dc in range(CD):
            tp_ps = psum_tp.tile([P, HWB], bf16, tag="tp")
            for c in range(CD):
                nc.tensor.matmul(
                    tp_ps[:, c * P : (c + 1) * P],
                    x_sb[:, c, dc * P : (dc + 1) * P],
                    ident,
                    is_transpose=True,
                )
            nc.vector.tensor_copy(xT[:, dc, :], tp_ps)

        qT8 = work_pool.tile([P, CD, HWB], fp8, tag="qT")
        for m in range(CD):
            mm_ps = psum_mm.tile([P, HWB], f32, tag="mm")
            for c in range(CD):
                nc.tensor.matmul(
                    mm_ps,
                    consts["wq_sb"][:, c, m * P : (m + 1) * P],
                    xT[:, c, :],
                    start=(c == 0),
                    stop=(c == CD - 1),
                )
            nc.scalar.activation(
                qT8[:, m, :], mm_ps, AFT.Identity,
                bias=consts["bq_sb"][:, m, None], scale=1.0,
            )

        kT8 = bt["kT8"]
        ex_bf = work_pool.tile([P, SC, HWB], bf16, tag="ex")
        for g in range(SC):
            sc_ps = psum_sc.tile([P, HWB], f32, tag="sc")
            nc.tensor.matmul(
                sc_ps, kT8[:, 0:2, g * P : (g + 1) * P], qT8[:, 0:2, :],
                start=True, stop=False, perf_mode=DR,
            )
            nc.tensor.matmul(
                sc_ps, kT8[:, 2:4, g * P : (g + 1) * P], qT8[:, 2:4, :],
                start=False, stop=True, perf_mode=DR,
            )
            nc.scalar.activation(
                ex_bf[:, g, :], sc_ps, AFT.Exp, bias=0.0, scale=SCALE
            )

        # u8 = fp8(ex - 1); split across ACT and DVE for engine balance
        u8 = work_pool.tile([P, SC, HWB], fp8, tag="u8")
        nc.scalar.activation(
            u8[:, 0:3, :], ex_bf[:, 0:3, :], AFT.Copy, bias=-1.0, scale=1.0
        )
        nc.vector.tensor_scalar_add(u8[:, 3:8, :], ex_bf[:, 3:8, :], -1.0)
        return u8

    def _block_B(b, blk, bt, u8):
        """attn @ V, denominator, normalization, store for block (b, blk)."""
        v8, colV2 = bt["v8"], bt["colV2"]
        out_sb = work_pool.tile([P, CD, D], bf16, tag="o")
        for h in range(CD):
            o_ps = psum_o.tile([P, D], f32, tag="o")
            d_ps = psum_den.tile([P, 2], f32, tag="den")
            # colV (hi+lo fp8 pair) opens the accumulation group
            nc.tensor.matmul(
                o_ps, ones12_8, colV2, start=True, stop=False, perf_mode=DR
            )
            for i in range(CD):
                lhs = u8[:, 2 * i : 2 * i + 2, h * P : (h + 1) * P]
                nc.tensor.matmul(
                    o_ps, lhs, v8[:, 2 * i : 2 * i + 2, :],
                    start=False, stop=(i == CD - 1), perf_mode=DR,
                )
                nc.tensor.matmul(
                    d_ps, lhs, ones22_8,
                    start=(i == 0), stop=(i == CD - 1), perf_mode=DR,
                )
            den_t = small_pool.tile([P, 1], f32, tag="den")
            nc.vector.tensor_scalar_add(den_t, d_ps[:, 0:1], float(S))
            rec = small_pool.tile([P, 1], f32, tag="rec")
            nc.vector.reciprocal(rec, den_t)
            nc.vector.tensor_scalar_mul(out_sb[:, h, :], o_ps, rec[:, 0:1])
        nc.sync.dma_start(
            out[b].rearrange("(t h p) n -> t p h n", p=P, h=CD)[blk], out_sb
        )

    # --- software-pipelined emission ----------------------------------
    pending = None
    for b in range(BL):
        bt = _prep_batch(b)
        for blk in range(NBLK):
            u8 = _block_A(b, blk, bt)
            if pending is not None:
                _block_B(*pending)
            pending = (b, blk, bt, u8)
    _block_B(*pending)


_NC_CACHE = None


def _get_nc():
    global _NC_CACHE
    if _NC_CACHE is None:
        _NC_CACHE = build_nc()
    return _NC_CACHE


def kernel(**inputs):
    x = np.asarray(inputs["x"], dtype=np.float32).astype(bf16_np)
    audio = np.asarray(inputs["audio_embed"], dtype=np.float32).astype(bf16_np)
    wq = np.asarray(inputs["Wq"], dtype=np.float32).astype(bf16_np)
    bq = np.ascontiguousarray(np.asarray(inputs["bq"], dtype=np.float32))
    wk = np.asarray(inputs["Wk"], dtype=np.float32).astype(bf16_np)
    bk = np.ascontiguousarray(np.asarray(inputs["bk"], dtype=np.float32))
    wv = np.asarray(inputs["Wv"], dtype=np.float32).astype(bf16_np)
    bv = np.ascontiguousarray(np.asarray(inputs["bv"], dtype=np.float32))

    nc = _get_nc()
    in_maps = []
    for i in range(N_CORES):
        in_maps.append(
            {
                "x": np.ascontiguousarray(x[i * BL : (i + 1) * BL]),
                "audio_embed": np.ascontiguousarray(audio[i * BL : (i + 1) * BL]),
                "Wq": wq,
                "bq": bq,
                "Wk": wk,
                "bk": bk,
                "Wv": wv,
                "bv": bv,
            }
        )
    res = run_bass_kernel_spmd(nc, in_maps, core_ids=list(range(N_CORES)))
    return np.concatenate(
        [res.results[i]["out"].astype(np.float32) for i in range(N_CORES)], axis=0
    )


# revision 7
# speedup vs baseline: 1.6596x; 1.0078x over previous
"""Cross-attention block kernel for Trainium2 (8 NeuronCores, data-parallel).

Computes, for full inputs:
    Q = x @ Wq + bq            [B, HW, D]
    K = a @ Wk + bk            [B, S, D]
    V = a @ Wv + bv            [B, S, D]
    out = softmax(Q K^T / sqrt(D)) @ V

Sharding: batch (B=16) split across 8 cores, 2 batches per core. Weights
replicated. No collectives needed.

Per-core kernel strategy:
  - All inputs host-cast to bf16 (halves DMA, transposes run 1.0 cyc/row,
    projection matmuls run at full PE rate like fp32r).
  - PE-transpose x and audio tiles into d-major layouts (xT, aT) in bf16.
  - qT8 = fp8e4(Wq^T @ xT + bq), kT8 = fp8e4(Wk^T @ aT + bk): ACT applies
    the bias straight out of PSUM and emits fp8e4.
  - v8 = fp8e4(aT^T @ Wv + bv) via DVE add out of PSUM.
  - scoresT[s, hw] = kT8^T @ qT8 with fp8 DoubleRow matmuls (2 k-tiles per
    instruction, 0.5 cycles/row): 4x the fp32r rate. ACT computes
    ex = exp(scale * scoresT) in bf16 (scores std ~0.33, exp is safe).
  - Softmax numerator uses mean-centering to keep fp8 quantization error
    small: weights ex = 1 + u with u = ex - 1 (|u| ~ 0.36), so
      out_num = colV + u8 @ V8,  colV[d] = sum_s V[s, d]  (exact-ish)
    u8 = fp8e4(ex - 1) carries quantization noise on u (~0.36), not on the
    full weight (~1.0): ~3x less error. colV is computed per batch as
    colA @ Wv + S*bv with colA = rowsum(aT), split into an fp8 hi+lo pair
    (residual splitting keeps the error ~0.06%) and accumulated into the
    output PSUM group via a single K=1 DoubleRow matmul.
  - Denominator den = S + sum_s u8 via a DoubleRow matmul against a ones
    column; DVE adds S, takes the reciprocal, and scales the output.
  - Output written as bf16 (harness tolerance 2e-2; bf16 adds ~0.2%).

The emission order is software-pipelined: the output phase (attn @ V) of
block i-1 is emitted after the compute phase of block i, so the PE never
waits for the same block's exp/u8 chain; x tiles are prefetched one block
ahead and audio one batch ahead.

fp8 error budget (measured): scores (Q,K in fp8e4) 1.18e-2, u8 0.63e-2,
V8 0.62e-2, bf16 inputs 0.27e-2 -> total ~1.46e-2 vs the 2e-2 gate.
"""

from contextlib import ExitStack

import numpy as np
import ml_dtypes

import concourse.bass as bass
import concourse.bacc as bacc
import concourse.mybir as mybir
import concourse.tile as tile
from concourse.bass_utils import run_bass_kernel_spmd
from concourse.masks import make_identity

P = 128
D = 512          # d_query == d_audio == d_out
CD = D // P      # 4 chunks of the feature dim
HW = 4096        # queries per batch
S = 1024         # keys per batch
SC = S // P      # 8 s-chunks
HWB = 512        # hw rows processed per block
NBLK = HW // HWB
B_FULL = 16
N_CORES = 8
BL = B_FULL // N_CORES  # 2 batches per core
SCALE = 1.0 / float(np.sqrt(D))

f32 = mybir.dt.float32
f32r = mybir.dt.float32r
bf16 = mybir.dt.bfloat16
fp8 = mybir.dt.float8e4
AFT = mybir.ActivationFunctionType
ALU = mybir.AluOpType
DR = mybir.MatmulPerfMode.DoubleRow

bf16_np = ml_dtypes.bfloat16


def build_nc():
    nc = bacc.Bacc("TRN2", target_bir_lowering=False, debug=False)

    x = nc.dram_tensor("x", [BL, HW, D], bf16, kind="ExternalInput").ap()
    audio = nc.dram_tensor("audio_embed", [BL, S, D], bf16, kind="ExternalInput").ap()
    wq = nc.dram_tensor("Wq", [D, D], bf16, kind="ExternalInput").ap()
    bq = nc.dram_tensor("bq", [D], f32, kind="ExternalInput").ap()
    wk = nc.dram_tensor("Wk", [D, D], bf16, kind="ExternalInput").ap()
    bk = nc.dram_tensor("bk", [D], f32, kind="ExternalInput").ap()
    wv = nc.dram_tensor("Wv", [D, D], bf16, kind="ExternalInput").ap()
    bv = nc.dram_tensor("bv", [D], f32, kind="ExternalInput").ap()
    out = nc.dram_tensor("out", [BL, HW, D], bf16, kind="ExternalOutput").ap()

    with tile.TileContext(nc) as tc:
        with ExitStack() as ctx:
            _body(ctx, tc, x, audio, wq, bq, wk, bk, wv, bv, out)

    nc.compile()
    return nc


def _body(ctx, tc, x, audio, wq, bq, wk, bk, wv, bv, out):
    nc = tc.nc

    const_pool = ctx.enter_context(tc.tile_pool(name="const", bufs=1))
    batch_pool = ctx.enter_context(tc.tile_pool(name="batch", bufs=2))
    work_pool = ctx.enter_context(tc.tile_pool(name="work", bufs=2))
    small_pool = ctx.enter_context(tc.tile_pool(name="small", bufs=4))
    psum_tp = ctx.enter_context(tc.tile_pool(name="ptp", bufs=1, space="PSUM"))
    psum_mm = ctx.enter_context(tc.tile_pool(name="pmm", bufs=2, space="PSUM"))
    psum_sc = ctx.enter_context(tc.tile_pool(name="psc", bufs=2, space="PSUM"))
    psum_o = ctx.enter_context(tc.tile_pool(name="po", bufs=2, space="PSUM"))
    psum_den = ctx.enter_context(tc.tile_pool(name="pden", bufs=1, space="PSUM"))

    # --- constants -----------------------------------------------------
    ident_f = const_pool.tile([P, P], f32)
    make_identity(nc, ident_f)
    ident = const_pool.tile([P, P], bf16)
    nc.vector.tensor_copy(ident, ident_f)

    # ones for the den DoubleRow matmul: rhs [K=128, 2 k-tiles, 2 cols]
    ones22_f = const_pool.tile([P, 2, 2], f32)
    nc.gpsimd.memset(ones22_f, 1.0)
    ones22_8 = const_pool.tile([P, 2, 2], fp8)
    nc.vector.tensor_copy(ones22_8, ones22_f)
    # ones lhsT for the K=1 colV DoubleRow matmul: [1, 2 k-tiles, 128]
    ones12_f = const_pool.tile([1, 2, P], f32)
    nc.gpsimd.memset(ones12_f, 1.0)
    ones12_8 = const_pool.tile([1, 2, P], fp8)
    nc.vector.tensor_copy(ones12_8, ones12_f)
    # ones row for the K=1 bv broadcast matmul (f32r chain)
    ones_row_f = const_pool.tile([1, P], f32)
    nc.gpsimd.memset(ones_row_f, 1.0)
    ones_row = const_pool.tile([1, P], f32r)
    nc.vector.tensor_copy(ones_row, ones_row_f)

    # Weight/bias loads are emitted lazily (after the first audio-half DMA)
    # so the first transposable input data leads the serial DMA queue.
    consts = {}

    def _load_consts():
        bv_row = const_pool.tile([1, D], f32)
        nc.sync.dma_start(bv_row, bv[None, :])
        bq_sb = const_pool.tile([P, CD], f32)
        nc.sync.dma_start(bq_sb, bq.rearrange("(c p) -> p c", p=P))
        bk_sb = const_pool.tile([P, CD], f32)
        nc.sync.dma_start(bk_sb, bk.rearrange("(c p) -> p c", p=P))
        wk_sb = const_pool.tile([P, CD, D], bf16)
        nc.sync.dma_start(wk_sb, wk.rearrange("(c p) n -> p c n", p=P))
        wv_sb = const_pool.tile([P, CD, D], bf16)
        nc.sync.dma_start(wv_sb, wv.rearrange("(c p) n -> p c n", p=P))
        # bv * S for the colV row (colV = colA @ Wv + S*bv)
        bv1024 = const_pool.tile([1, D], f32)
        nc.vector.tensor_scalar_mul(bv1024, bv_row, float(S))
        # bv broadcast to all 128 partitions via a K=1 outer-product matmul
        bv_row_r = const_pool.tile([1, D], f32r)
        nc.vector.tensor_copy(bv_row_r, bv_row)
        bv_ps = psum_mm.tile([P, D], f32, tag="mm")
        nc.tensor.matmul(bv_ps, ones_row, bv_row_r, start=True, stop=True)
        bv_bc = const_pool.tile([P, D], f32)
        nc.vector.tensor_copy(bv_bc, bv_ps)
        consts.update(wk_sb=wk_sb, wv_sb=wv_sb,
                      bq_sb=bq_sb, bk_sb=bk_sb, bv_bc=bv_bc, bv1024=bv1024)

    def _load_wq():
        wq_sb = const_pool.tile([P, CD, D], bf16)
        nc.sync.dma_start(wq_sb, wq.rearrange("(c p) n -> p c n", p=P))
        consts.update(wq_sb=wq_sb)

    def _dma_x(b, blk):
        t = work_pool.tile([P, CD, D], bf16, tag="x")
        nc.sync.dma_start(
            t, x[b].rearrange("(t c p) n -> t p c n", p=P, c=CD)[blk]
        )
        return t

    def _dma_audio_half(b, half, split=False):
        t = work_pool.tile([P, CD, D], bf16, tag="ah")
        src = audio[b].rearrange("(t c p) n -> t p c n", p=P, c=CD)[half]
        if split:
            # per-c-chunk DMAs so the first transpose starts ~4x sooner
            for c in range(CD):
                nc.sync.dma_start(t[:, c, :], src[:, c, :])
        else:
            nc.sync.dma_start(t, src)
        return t

    x_pre = {}
    audio_pre = {}

    def _prep_batch(b):
        """Audio transpose, K^T (fp8), V (fp8), colV pair for batch b."""
        aT = batch_pool.tile([P, CD, S], bf16, tag="aT")
        kT8 = batch_pool.tile([P, CD, S], fp8, tag="kT")
        v8 = batch_pool.tile([P, SC, D], fp8, tag="v")
        for half in range(2):
            a_half = audio_pre.pop((b, half), None)
            if a_half is None:
                a_half = _dma_audio_half(b, half, split=(b == 0 and half == 0))
            if b == 0 and half == 0:
                _load_consts()
                x_pre[(0, 0)] = _dma_x(0, 0)
                _load_wq()
            for dc in range(CD):
                tp_ps = psum_tp.tile([P, HWB], bf16, tag="tp")
                for c in range(CD):
                    nc.tensor.matmul(
                        tp_ps[:, c * P : (c + 1) * P],
                        a_half[:, c, dc * P : (dc + 1) * P],
                        ident,
                        is_transpose=True,
                    )
                nc.vector.tensor_copy(aT[:, dc, half * 512 : (half + 1) * 512], tp_ps)

            for m in range(CD):
                mm_ps = psum_mm.tile([P, 512], f32, tag="mm")
                for c in range(CD):
                    nc.tensor.matmul(
                        mm_ps,
                        consts["wk_sb"][:, c, m * P : (m + 1) * P],
                        aT[:, c, half * 512 : (half + 1) * 512],
                        start=(c == 0),
                        stop=(c == CD - 1),
                    )
                nc.scalar.activation(
                    kT8[:, m, half * 512 : (half + 1) * 512],
                    mm_ps,
                    AFT.Identity,
                    bias=consts["bk_sb"][:, m, None],
                    scale=1.0,
                )

            for g in range(half * 4, half * 4 + 4):
                mm_ps = psum_mm.tile([P, D], f32, tag="mm")
                for c in range(CD):
                    nc.tensor.matmul(
                        mm_ps,
                        aT[:, c, g * P : (g + 1) * P],
                        consts["wv_sb"][:, c, :],
                        start=(c == 0),
                        stop=(c == CD - 1),
                    )
                nc.vector.tensor_add(v8[:, g, :], mm_ps, consts["bv_bc"])

        # colV[d] = sum_s V[s, d] = colA @ Wv + S*bv, split to fp8 hi+lo
        colA_f = small_pool.tile([P, CD], f32, tag="colA")
        nc.vector.tensor_reduce(
            colA_f, aT, axis=mybir.AxisListType.X, op=ALU.add
        )
        colA_b = small_pool.tile([P, CD], bf16, tag="colAb")
        nc.vector.tensor_copy(colA_b, colA_f)
        cv_ps = psum_sc.tile([P, HWB], f32, tag="sc")
        for c in range(CD):
            nc.tensor.matmul(
                cv_ps[0:1, :],
                colA_b[:, c, None],
                consts["wv_sb"][:, c, :],
                start=(c == 0),
                stop=(c == CD - 1),
            )
        colV_f = small_pool.tile([1, D], f32, tag="colVf")
        nc.vector.tensor_add(colV_f, cv_ps[0:1, :], consts["bv1024"])
        colV2 = batch_pool.tile([1, 2, D], fp8, tag="colV")
        nc.vector.tensor_copy(colV2[:, 0, :], colV_f)
        nc.vector.tensor_sub(colV2[:, 1, :], colV_f, colV2[:, 0, :])
        return dict(aT=aT, kT8=kT8, v8=v8, colV2=colV2)

    def _block_A(b, blk, bt):
        """Transposes, Q^T (fp8), scores, exp, u8 for block (b, blk)."""
        # prefetch the next x block (or next batch's audio) while computing
        nxt = (b, blk + 1) if blk + 1 < NBLK else (b + 1, 0)
        if nxt[0] < BL and nxt not in x_pre:
            x_pre[nxt] = _dma_x(*nxt)
        if blk == NBLK - 2 and b + 1 < BL:
            for half in range(2):
                audio_pre[(b + 1, half)] = _dma_audio_half(b + 1, half)

        x_sb = x_pre.pop((b, blk))

        xT = work_pool.tile([P, CD, HWB], bf16, tag="xT")
        for dc in range(CD):
            tp_ps = psum_tp.tile([P, HWB], bf16, tag="tp")
            for c in range(CD):
                nc.tensor.matmul(
                    tp_ps[:, c * P : (c + 1) * P],
                    x_sb[:, c, dc * P : (dc + 1) * P],
                    ident,
                    is_transpose=True,
                )
            nc.vector.tensor_copy(xT[:, dc, :], tp_ps)

        qT8 = work_pool.tile([P, CD, HWB], fp8, tag="qT")
        for m in range(CD):
            mm_ps = psum_mm.tile([P, HWB], f32, tag="mm")
            for c in range(CD):
                nc.tensor.matmul(
                    mm_ps,
                    consts["wq_sb"][:, c, m * P : (m + 1) * P],
                    xT[:, c, :],
                    start=(c == 0),
                    stop=(c == CD - 1),
                )
            nc.scalar.activation(
                qT8[:, m, :], mm_ps, AFT.Identity,
                bias=consts["bq_sb"][:, m, None], scale=1.0,
            )

        kT8 = bt["kT8"]
        ex_bf = work_pool.tile([P, SC, HWB], bf16, tag="ex")
        for g in range(SC):
            sc_ps = psum_sc.tile([P, HWB], f32, tag="sc")
            nc.tensor.matmul(
                sc_ps, kT8[:, 0:2, g * P : (g + 1) * P], qT8[:, 0:2, :],
                start=True, stop=False, perf_mode=DR,
            )
            nc.tensor.matmul(
                sc_ps, kT8[:, 2:4, g * P : (g + 1) * P], qT8[:, 2:4, :],
                start=False, stop=True, perf_mode=DR,
            )
            nc.scalar.activation(
                ex_bf[:, g, :], sc_ps, AFT.Exp, bias=0.0, scale=SCALE
            )

        # u8 = fp8(ex - 1); split across ACT and DVE for engine balance
        u8 = work_pool.tile([P, SC, HWB], fp8, tag="u8")
        nc.scalar.activation(
            u8[:, 0:2, :], ex_bf[:, 0:2, :], AFT.Copy, bias=-1.0, scale=1.0
        )
        nc.vector.tensor_scalar_add(u8[:, 2:8, :], ex_bf[:, 2:8, :], -1.0)
        return u8

    def _block_B(b, blk, bt, u8):
        """attn @ V, denominator, normalization, store for block (b, blk)."""
        v8, colV2 = bt["v8"], bt["colV2"]
        out_sb = work_pool.tile([P, CD, D], bf16, tag="o")
        # denominators for all four hw-chunks first, one reciprocal chain
        d_all = psum_den.tile([P, CD, 2], f32, tag="den")
        for h in range(CD):
            for i in range(CD):
                nc.tensor.matmul(
                    d_all[:, h, :],
                    u8[:, 2 * i : 2 * i + 2, h * P : (h + 1) * P],
                    ones22_8,
                    start=(i == 0), stop=(i == CD - 1), perf_mode=DR,
                )
        den_all = small_pool.tile([P, CD], f32, tag="den")
        nc.vector.tensor_scalar_add(den_all, d_all[:, :, 0], float(S))
        rec_all = small_pool.tile([P, CD], f32, tag="rec")
        nc.vector.reciprocal(rec_all, den_all)
        for h in range(CD):
            o_ps = psum_o.tile([P, D], f32, tag="o")
            # colV (hi+lo fp8 pair) opens the accumulation group
            nc.tensor.matmul(
                o_ps, ones12_8, colV2, start=True, stop=False, perf_mode=DR
            )
            for i in range(CD):
                nc.tensor.matmul(
                    o_ps,
                    u8[:, 2 * i : 2 * i + 2, h * P : (h + 1) * P],
                    v8[:, 2 * i : 2 * i + 2, :],
                    start=False, stop=(i == CD - 1), perf_mode=DR,
                )
            nc.vector.tensor_scalar_mul(out_sb[:, h, :], o_ps, rec_all[:, h : h + 1])
        nc.sync.dma_start(
            out[b].rearrange("(t h p) n -> t p h n", p=P, h=CD)[blk], out_sb
        )

    # --- software-pipelined emission ----------------------------------
    pending = None
    for b in range(BL):
        bt = _prep_batch(b)
        for blk in range(NBLK):
            u8 = _block_A(b, blk, bt)
            if pending is not None:
                _block_B(*pending)
            pending = (b, blk, bt, u8)
    _block_B(*pending)


_NC_CACHE = None


def _get_nc():
    global _NC_CACHE
    if _NC_CACHE is None:
        _NC_CACHE = build_nc()
    return _NC_CACHE


def kernel(**inputs):
    x = np.asarray(inputs["x"], dtype=np.float32).astype(bf16_np)
    audio = np.asarray(inputs["audio_embed"], dtype=np.float32).astype(bf16_np)
    wq = np.asarray(inputs["Wq"], dtype=np.float32).astype(bf16_np)
    bq = np.ascontiguousarray(np.asarray(inputs["bq"], dtype=np.float32))
    wk = np.asarray(inputs["Wk"], dtype=np.float32).astype(bf16_np)
    bk = np.ascontiguousarray(np.asarray(inputs["bk"], dtype=np.float32))
    wv = np.asarray(inputs["Wv"], dtype=np.float32).astype(bf16_np)
    bv = np.ascontiguousarray(np.asarray(inputs["bv"], dtype=np.float32))

    nc = _get_nc()
    in_maps = []
    for i in range(N_CORES):
        in_maps.append(
            {
                "x": np.ascontiguousarray(x[i * BL : (i + 1) * BL]),
                "audio_embed": np.ascontiguousarray(audio[i * BL : (i + 1) * BL]),
                "Wq": wq,
                "bq": bq,
                "Wk": wk,
                "bk": bk,
                "Wv": wv,
                "bv": bv,
            }
        )
    res = run_bass_kernel_spmd(nc, in_maps, core_ids=list(range(N_CORES)))
    return np.concatenate(
        [res.results[i]["out"].astype(np.float32) for i in range(N_CORES)], axis=0
    )
